# revision 3
# baseline (speedup 1.0000x reference)
"""Gemma3 single-token decode on 8 trn2 NeuronCores (tensor-parallel SPMD).

Sharding: attention by head (pairs of cores compute the same head redundantly,
Wo pre-scaled by 0.5 so the 8-way AllReduce sums correctly); FFN 8-way over the
FF dim; lm_head 8-way over vocab with host-side final argmax; KV cache sliced
to the live prefix and replicated; norms computed on every core.

All weights and the KV cache are shipped as fp8 e4m3 scaled by 32 (host-side
exponent add) to halve wire bytes; activations stay bf16/f32 and every matmul
accumulates in f32 PSUM, with the 1/32 (or 1/1024 for weight*weight chains)
descale folded into the PSUM->SBUF copies.  Inputs are staged per-core via
jax.make_array_from_single_device_arrays from a background thread so transfer
overlaps host-side prep; no global host concat.
"""
import sys, os
sys.path.insert(0, '/opt/trn_rl_repo')
import threading
import queue as _queue
import numpy as np
import ml_dtypes

import concourse.bass as bass
import concourse.bacc as bacc
import concourse.mybir as mybir
import concourse.tile as tile

L, HID, NCH, D, H, FF, VOCAB = 12, 1152, 9, 256, 4, 6912, 64000
FSH = FF // 8            # 864 ffn rows per core
VS = VOCAB // 8          # 8000 vocab rows per core
SEFF, T = 1024, 8        # live kv prefix (pos=1000 -> 1024), 8 s-tiles
SCALE, EPS = 256.0 ** -0.5, 1e-6
NC_ = 8
F32 = mybir.dt.float32
BF16 = mybir.dt.bfloat16
E4 = mybir.dt.float8e4
AF = mybir.ActivationFunctionType
X_AX = mybir.AxisListType.X

E4NP = ml_dtypes.float8_e4m3
WS = 32.0                # fp8 weight pre-scale (power of two)
_PROG_CACHE = {}

try:
    import jax
    jax.config.update('jax_compilation_cache_dir', '/tmp/jax_pcache')
    jax.config.update('jax_persistent_cache_min_compile_time_secs', 0)
    jax.config.update('jax_persistent_cache_min_entry_size_bytes', 0)
except Exception:
    pass


def _build():
    nc = bacc.Bacc("TRN2", target_bir_lowering=False, debug=False, num_devices=NC_)
    _eps_t = nc.alloc_sbuf_tensor("const-eps", [128, 1], F32)
    nc.gpsimd.memset(_eps_t.ap(), EPS)
    nc.const_aps.aps[(F32, EPS)] = _eps_t.ap()
    nc.all_engine_barrier()

    def dI(n, sh, dt=F32):
        return nc.dram_tensor(n, sh, dt, kind="ExternalInput").ap()

    h0row = dI("h0row", [1, HID])
    cs = dI("cs", [1, 1024])
    mcol = dI("mcol", [128, 40])
    wqkv = dI("wqkv", [L, 3, 128, 2304], E4)
    wo = dI("wo", [L, 128, 2, HID], E4)
    ktd = dI("kt", [L, 128, 2, SEFF], E4)
    vcd = dI("vc", [L, 128, T, D], E4)
    wgd = dI("wg", [L, 3, 128, 2592], E4)
    wud = dI("wu", [L, 3, 128, 2592], E4)
    wdd = dI("wd", [L, 128, 7, HID], E4)
    lmd = dI("lm", [NCH, 128, VS], E4)
    logits = nc.dram_tensor("logits", [1, VS], F32, kind="ExternalOutput").ap()

    with tile.TileContext(nc) as tc, \
         tc.tile_pool(name="const", bufs=1) as Pc, \
         tc.tile_pool(name="wqkv", bufs=2) as Pwq, \
         tc.tile_pool(name="wo", bufs=1) as Pwo, \
         tc.tile_pool(name="kt", bufs=1) as Pkt, \
         tc.tile_pool(name="vc", bufs=1) as Pvc, \
         tc.tile_pool(name="wg", bufs=2) as Pwg, \
         tc.tile_pool(name="wu", bufs=2) as Pwu, \
         tc.tile_pool(name="wd", bufs=2) as Pwd, \
         tc.tile_pool(name="lm", bufs=2) as Plm, \
         tc.tile_pool(name="act", bufs=2) as Pa, \
         tc.tile_pool(name="row", bufs=3) as Pr, \
         tc.tile_pool(name="ps", bufs=2, space="PSUM") as Pp, \
         tc.tile_pool(name="dram", bufs=2, space="DRAM") as Pd:

        MM = nc.tensor.matmul
        one_f = Pc.tile([1, 1], F32, tag="onef")
        nc.vector.memset(one_f[:], 1.0)
        one_w = Pc.tile([1, 1], BF16, tag="onew")
        nc.vector.memset(one_w[:], 1.0)
        ones_cf = Pc.tile([128, 1], F32, tag="ocf")
        nc.vector.memset(ones_cf[:], 1.0)
        cs_t = Pc.tile([1, 1024], F32, tag="cs")
        nc.sync.dma_start(out=cs_t[:], in_=cs[:])
        mc = Pc.tile([128, 40], F32, tag="mc")
        nc.sync.dma_start(out=mc[:], in_=mcol[:])
        ADDM, VM, VMU, UM1, UMF = (mc[:, 8 * i:8 * i + 8] for i in range(5))

        def cast_col(src_t, tag):
            w = Pa.tile([128, NCH], BF16, tag=tag)
            nc.vector.tensor_copy(w[:], src_t[:])
            return w

        def columnize(row_ap, n, one_t, PS, base):
            ps = PS[:, base:base + n]
            for j in range(n):
                MM(ps[:, j:j + 1], row_ap[0:1, j * 128:(j + 1) * 128], one_t[:],
                   start=True, stop=True)
            return ps

        def rms_col(h_t, tag, PS, base):
            sq = Pa.tile([128, NCH], F32, tag="sq")
            nc.vector.tensor_mul(sq[:], h_t[:], h_t[:])
            MM(PS[0:1, base:base + NCH], ones_cf[:], sq[:], start=True, stop=True)
            st = Pa.tile([1, 4], F32, tag="rmsst")
            nc.vector.reduce_sum(st[0:1, 0:1], PS[0:1, base:base + NCH], axis=X_AX)
            nc.scalar.activation(st[0:1, 1:2], st[0:1, 0:1], AF.Sqrt,
                                 bias=EPS, scale=1.0 / HID)
            nc.vector.reciprocal(st[0:1, 2:3], st[0:1, 1:2])
            rb = Pa.tile([128, 1], F32, tag="rb")
            nc.gpsimd.partition_broadcast(rb[:], st[0:1, 2:3])
            x = Pa.tile([128, NCH], F32, tag=tag)
            nc.vector.tensor_scalar_mul(x[:], h_t[:], rb[:])
            return x

        def resid_add(h_t, row_t, PS):
            st = Pa.tile([1, 4], F32, tag="rmsst")
            scr = Pr.tile([1, HID], F32, tag="r1152")
            nc.scalar.activation(scr[:], row_t[:], AF.Square,
                                 accum_out=st[0:1, 0:1])
            nc.scalar.activation(st[0:1, 1:2], st[0:1, 0:1], AF.Sqrt,
                                 bias=EPS, scale=1.0 / HID)
            nc.vector.reciprocal(st[0:1, 2:3], st[0:1, 1:2])
            rb = Pa.tile([128, 1], F32, tag="rb")
            nc.gpsimd.partition_broadcast(rb[:], st[0:1, 2:3])
            pc = columnize(row_t, NCH, one_f, PS, 64)
            tmp = Pa.tile([128, NCH], F32, tag="tmph")
            nc.vector.tensor_scalar_mul(tmp[:], pc[:], rb[:])
            hn = Pa.tile([128, NCH], F32, tag="h")
            nc.vector.tensor_add(hn[:], h_t[:], tmp[:])
            return hn

        def all_reduce(row_t):
            bin_ = Pd.tile([1, HID], F32, tag="arin")
            bout = Pd.tile([1, HID], F32, tag="arout")
            nc.gpsimd.dma_start(out=bin_[:], in_=row_t[:])
            nc.gpsimd.collective_compute(
                "AllReduce", mybir.AluOpType.add,
                replica_groups=[list(range(NC_))],
                ins=[bin_.opt()], outs=[bout.opt()])
            ar = Pr.tile([1, HID], F32, tag="r1152")
            nc.gpsimd.dma_start(out=ar[:], in_=bout[:])
            return ar

        # h0: [1,1152] row -> column layout
        h0r = Pr.tile([1, HID], F32, tag="r1152")
        nc.sync.dma_start(out=h0r[:], in_=h0row[:])
        PS = Pp.tile([128, 512], F32, tag="psmall")
        pc0 = columnize(h0r, NCH, one_f, PS, 64)
        h = Pa.tile([128, NCH], F32, tag="h")
        nc.scalar.activation(h[:], pc0[:], AF.Copy)

        for l in range(L):
            # ---- attention ----
            PS = Pp.tile([128, 512], F32, tag="psmall")
            x = rms_col(h, "x", PS, 0)
            xw = cast_col(x, "xw")
            pqkv = Pp.tile([1, 1152], F32, tag="pbig")   # 32*[q|k|v]
            for g in range(3):
                wt = Pwq.tile([128, 2304], E4, tag="wqkv")
                nc.sync.dma_start(out=wt[:], in_=wqkv[l, g])
                for ci in range(3):
                    c = g * 3 + ci
                    for n0, ln in ((0, 512), (512, 256)):
                        MM(pqkv[0:1, n0:n0 + ln], xw[:, c:c + 1],
                           wt[:, ci * 768 + n0: ci * 768 + n0 + ln],
                           start=(c == 0), stop=(c == 8))
            # q/k rms over D (rows on partition 0); the x32 scale cancels in
            # q/rms(q), up to the negligible EPS shift.
            st = Pa.tile([1, 6], F32, tag="qkst")
            scr = Pr.tile([1, 256], F32, tag="r256")
            nc.scalar.activation(scr[:], pqkv[0:1, 0:256], AF.Square,
                                 accum_out=st[0:1, 0:1])
            scr2 = Pr.tile([1, 256], F32, tag="r256")
            nc.scalar.activation(scr2[:], pqkv[0:1, 256:512], AF.Square,
                                 accum_out=st[0:1, 1:2])
            nc.scalar.activation(st[0:1, 2:3], st[0:1, 0:1], AF.Sqrt,
                                 bias=EPS, scale=1.0 / D)
            nc.scalar.activation(st[0:1, 3:4], st[0:1, 1:2], AF.Sqrt,
                                 bias=EPS, scale=1.0 / D)
            nc.vector.reciprocal(st[0:1, 4:5], st[0:1, 2:3])
            nc.vector.reciprocal(st[0:1, 5:6], st[0:1, 3:4])
            cof = 512 if ((l + 1) % 6 == 0) else 0
            cosr = cs_t[0:1, cof:cof + 256]
            sinr = cs_t[0:1, cof + 256:cof + 512]

            def rope(off, rinv, tag):
                t1 = Pr.tile([1, 256], F32, tag="ropet")
                nc.vector.tensor_mul(t1[:], pqkv[0:1, off:off + 256], cosr)
                sw = Pr.tile([1, 256], F32, tag="ropes")
                nc.vector.tensor_copy(sw[0:1, 0:128], pqkv[0:1, off + 128:off + 256])
                nc.vector.tensor_copy(sw[0:1, 128:256], pqkv[0:1, off:off + 128])
                nc.vector.tensor_mul(sw[:], sw[:], sinr)
                nc.vector.tensor_add(t1[:], t1[:], sw[:])
                out = Pr.tile([1, 256], F32, tag=tag)
                nc.vector.tensor_scalar_mul(out[:], t1[:], rinv)
                return out

            qr = rope(0, st[0:1, 4:5], "qr")
            kr = rope(256, st[0:1, 5:6], "kr")
            # columnize q,k -> [128,2] each (bf16, unscaled)
            pqc = PS[:, 88:92]
            for j in range(2):
                MM(pqc[:, j:j + 1], qr[0:1, j * 128:(j + 1) * 128], one_f[:],
                   start=True, stop=True)
                MM(pqc[:, 2 + j:3 + j], kr[0:1, j * 128:(j + 1) * 128], one_f[:],
                   start=True, stop=True)
            qkc = Pa.tile([128, 4], BF16, tag="qkc")
            nc.scalar.activation(qkc[:], pqc[:], AF.Copy)

            # scores^T [128, T] (s = t*128 + r); cache is x32 -> psc = 32*scores
            kt_t = Pkt.tile([128, 2, SEFF], E4, tag="kt")
            nc.sync.dma_start(out=kt_t[:], in_=ktd[l])
            psc = PS[:, 80:88]
            for t_ in range(T):
                for c in range(2):
                    MM(psc[:, t_:t_ + 1],
                       kt_t[:, c, t_ * 128: t_ * 128 + 128],
                       qkc[:, c:c + 1], start=(c == 0), stop=(c == 1))
            # qk_new = q . k_new (unscaled bf16) -> x32 to match psc
            pqk = PS[0:1, 18:48]
            for c in range(2):
                MM(pqk[0:1, 10:11], qkc[:, c:c + 1], qkc[:, 2 + c:3 + c],
                   start=(c == 0), stop=(c == 1))
            qks = Pa.tile([1, 1], F32, tag="qks")
            nc.scalar.activation(qks[:], pqk[0:1, 10:11], AF.Copy, scale=WS)
            bq = Pa.tile([128, 1], F32, tag="bq")
            nc.gpsimd.partition_broadcast(bq[:], qks[:])
            # fix scores at s=p, scale (incl 1/32 descale), mask, clamp, exp
            sc1 = Pa.tile([128, T], F32, tag="sc1")
            nc.vector.tensor_mul(sc1[:], psc[:], UM1)
            sc2 = Pa.tile([128, T], F32, tag="sc2")
            nc.vector.tensor_scalar_mul(sc2[:], UMF, bq[:])
            nc.vector.tensor_add(sc1[:], sc1[:], sc2[:])
            nc.vector.tensor_scalar_mul(sc1[:], sc1[:], float(SCALE / WS))
            nc.vector.tensor_add(sc1[:], sc1[:], ADDM)
            nc.vector.tensor_scalar_max(sc1[:], sc1[:], -30.0)
            probs = Pa.tile([128, T], F32, tag="probs")
            nc.scalar.activation(probs[:], sc1[:], AF.Exp)
            # denominator and p_at_update (f32)
            pmf = Pa.tile([128, T], F32, tag="pmf")
            nc.vector.tensor_mul(pmf[:], probs[:], VM)
            puf = Pa.tile([128, T], F32, tag="puf")
            nc.vector.tensor_mul(puf[:], probs[:], UMF)
            MM(pqk[0:1, 0:8], ones_cf[:], pmf[:], start=True, stop=True)
            psums = Pa.tile([1, 8], F32, tag="psums")
            nc.scalar.activation(psums[:], pqk[0:1, 0:8], AF.Copy)
            MM(pqk[0:1, 8:10], ones_cf[:], puf[:, 0:2], start=True, stop=False)
            MM(pqk[0:1, 8:10], ones_cf[:], puf[:, 2:4], start=False, stop=False)
            MM(pqk[0:1, 8:10], ones_cf[:], puf[:, 4:6], start=False, stop=False)
            MM(pqk[0:1, 8:10], ones_cf[:], puf[:, 6:8], start=False, stop=True)
            dn = Pa.tile([1, 4], F32, tag="dn")
            nc.vector.reduce_sum(dn[0:1, 0:1], psums[0:1, 0:8], axis=X_AX)
            nc.vector.reciprocal(dn[0:1, 1:2], dn[0:1, 0:1])
            nc.vector.reduce_sum(dn[0:1, 2:3], pqk[0:1, 8:10], axis=X_AX)
            # o = (probs_masked @ V + pu*v_new) / den  (V cache x32, v_new x32)
            pmv = Pa.tile([128, T], BF16, tag="pmv")
            nc.vector.tensor_mul(pmv[:], probs[:], VMU)
            vc_t = Pvc.tile([128, T, D], E4, tag="vc")
            nc.sync.dma_start(out=vc_t[:], in_=vcd[l])
            po = PS[0:1, 128:384]
            for t_ in range(T):
                MM(po[0:1, 0:256], pmv[:, t_:t_ + 1], vc_t[:, t_, :],
                   start=(t_ == 0), stop=(t_ == T - 1))
            vv = Pr.tile([1, 256], F32, tag="vv")
            nc.vector.tensor_scalar_mul(vv[:], pqkv[0:1, 512:768], dn[0:1, 2:3])
            ofin = Pr.tile([1, 256], F32, tag="ofin")
            nc.vector.tensor_add(ofin[:], po[0:1, 0:256], vv[:])
            nc.vector.tensor_scalar_mul(ofin[:], ofin[:], dn[0:1, 1:2])
            # Wo partial: wo is x16 (x32 fp8 scale * 0.5 redundancy), o is x32
            poc = PS[:, 92:96]
            for j in range(2):
                MM(poc[:, j:j + 1], ofin[0:1, j * 128:(j + 1) * 128], one_f[:],
                   start=True, stop=True)
            ocol = Pa.tile([128, 2], BF16, tag="ocol")
            nc.scalar.activation(ocol[:], poc[:, 92 - 92:94 - 92], AF.Copy)
            wo_t = Pwo.tile([128, 2, HID], E4, tag="wo")
            nc.sync.dma_start(out=wo_t[:], in_=wo[l])
            prow = Pp.tile([1, HID], F32, tag="pbig")
            for c in range(2):
                for n0, ln in ((0, 512), (512, 512), (1024, 128)):
                    MM(prow[0:1, n0:n0 + ln], ocol[:, c:c + 1],
                       wo_t[:, c, n0: n0 + ln],
                       start=(c == 0), stop=(c == 1))
            arow = Pr.tile([1, HID], F32, tag="r1152")
            nc.scalar.activation(arow[:], prow[0:1, :], AF.Copy,
                                 scale=1.0 / (WS * WS))
            ar1 = all_reduce(arow)
            h = resid_add(h, ar1, PS)

            # ---- ffn ----
            x2 = rms_col(h, "x2", PS, 9)
            x2w = cast_col(x2, "x2w")
            pg = Pp.tile([1, FSH], F32, tag="pbig", padded_shape=[1, HID])
            pu_ = Pp.tile([1, FSH], F32, tag="pbig", padded_shape=[1, HID])
            for g in range(3):
                wg_t = Pwg.tile([128, 2592], E4, tag="wg")
                nc.sync.dma_start(out=wg_t[:], in_=wgd[l, g])
                wu_t = Pwu.tile([128, 2592], E4, tag="wu")
                nc.sync.dma_start(out=wu_t[:], in_=wud[l, g])
                for ci in range(3):
                    c = g * 3 + ci
                    for n0, ln in ((0, 512), (512, 352)):
                        MM(pg[0:1, n0:n0 + ln], x2w[:, c:c + 1],
                           wg_t[:, ci * FSH + n0: ci * FSH + n0 + ln],
                           start=(c == 0), stop=(c == 8))
                        MM(pu_[0:1, n0:n0 + ln], x2w[:, c:c + 1],
                           wu_t[:, ci * FSH + n0: ci * FSH + n0 + ln],
                           start=(c == 0), stop=(c == 8))
            # pg/pu are x32: descale inside gelu, keep up x32 in the product
            gact = Pr.tile([1, FSH], F32, tag="gact")
            nc.scalar.activation(gact[:], pg[0:1, :], AF.Gelu_apprx_tanh,
                                 scale=1.0 / WS)
            prod = Pr.tile([1, 896], BF16, tag="prod")
            nc.vector.memset(prod[0:1, FSH:896], 0.0)
            nc.vector.tensor_mul(prod[0:1, 0:FSH], gact[:], pu_[0:1, :])
            pcd = columnize(prod, 7, one_w, PS, 64)
            pdc = Pa.tile([128, 7], BF16, tag="pdc")
            nc.scalar.activation(pdc[:], pcd[:], AF.Copy)
            pf = Pp.tile([1, HID], F32, tag="pbig")
            for s_ in range(4):
                nf = 2 if s_ < 3 else 1
                wd_t = Pwd.tile([128, nf, HID], E4, tag="wd")
                nc.sync.dma_start(out=wd_t[:],
                                  in_=wdd[l][:, 2 * s_:2 * s_ + nf, :])
                for fi in range(nf):
                    fc = 2 * s_ + fi
                    for n0, ln in ((0, 512), (512, 512), (1024, 128)):
                        MM(pf[0:1, n0:n0 + ln], pdc[:, fc:fc + 1],
                           wd_t[:, fi, n0: n0 + ln],
                           start=(fc == 0), stop=(fc == 6))
            frow = Pr.tile([1, HID], F32, tag="r1152")
            nc.scalar.activation(frow[:], pf[0:1, :], AF.Copy,
                                 scale=1.0 / (WS * WS))
            ar2 = all_reduce(frow)
            h = resid_add(h, ar2, PS)

        # ---- final norm + lm_head (vocab shard) ----
        PSf = Pp.tile([128, 512], F32, tag="psmall")
        xf = rms_col(h, "xf", PSf, 0)
        xfw = cast_col(xf, "xfw")
        for qt in range(4):
            pva = Pp.tile([1, HID], F32, tag="pbig", name=f"pva{qt}")
            pvb = Pp.tile([1, HID], F32, tag="pbig", name=f"pvb{qt}")
            regs = [pva[0:1, 0:500], pva[0:1, 512:1012],
                    pvb[0:1, 0:500], pvb[0:1, 512:1012]]
            for c in range(NCH):
                lm_t = Plm.tile([128, 2000], E4, tag="lm")
                nc.sync.dma_start(out=lm_t[:],
                                  in_=lmd[c, :, qt * 2000:(qt + 1) * 2000])
                for vi in range(4):
                    MM(regs[vi], xfw[:, c:c + 1],
                       lm_t[:, vi * 500:(vi + 1) * 500],
                       start=(c == 0), stop=(c == NCH - 1))
            for vi in range(4):
                vg = qt * 4 + vi
                lrow = Pr.tile([1, 500], F32, tag="lrow")
                nc.scalar.activation(lrow[:], regs[vi], AF.Copy, scale=1.0 / WS)
                nc.gpsimd.dma_start(out=logits[0:1, vg * 500:(vg + 1) * 500],
                                    in_=lrow[:])

    nc.compile()
    return nc


def _get_prog():
    if "prog" not in _PROG_CACHE:
        _PROG_CACHE["prog"] = _build()
    return _PROG_CACHE["prog"]


# ---------------------------------------------------------------------------
# Custom PJRT runner: like bass2jax.run_bass_via_pjrt but takes pre-sharded
# per-core device arrays (no host-side global concat) and caches the jit.
# ---------------------------------------------------------------------------

def _get_runner(nc):
    if "runner" in _PROG_CACHE:
        return _PROG_CACHE["runner"]
    import jax
    from jax.sharding import Mesh, PartitionSpec, NamedSharding
    from jax.experimental.shard_map import shard_map
    from concourse import bass2jax
    bass2jax.install_neuronx_cc_hook()

    partition_name = (nc.partition_id_tensor.name
                      if nc.partition_id_tensor else None)
    in_names, out_names, out_avals, zero_outs = [], [], [], []
    for alloc in nc.m.functions[0].allocations:
        if not isinstance(alloc, mybir.MemoryLocationSet):
            continue
        name = alloc.memorylocations[0].name
        if alloc.kind == "ExternalInput":
            if name != partition_name:
                in_names.append(name)
        elif alloc.kind == "ExternalOutput":
            shape = tuple(alloc.tensor_shape)
            dtype = mybir.dt.np(alloc.dtype)
            out_names.append(name)
            out_avals.append(jax.core.ShapedArray(shape, dtype))
            zero_outs.append(np.zeros(shape, dtype))
    n_params = len(in_names)
    all_names = in_names + out_names
    if partition_name is not None:
        all_names.append(partition_name)
    donate = tuple(range(n_params, n_params + len(out_names)))

    def _body(*args):
        operands = list(args)
        if partition_name is not None:
            operands.append(bass2jax.partition_id_tensor())
        outs = bass2jax._bass_exec_p.bind(
            *operands,
            out_avals=tuple(out_avals),
            in_names=tuple(all_names),
            out_names=tuple(out_names),
            lowering_input_output_aliases=(),
            sim_require_finite=True,
            sim_require_nnan=True,
            nc=nc,
        )
        return tuple(outs)

    devices = jax.devices()[:NC_]
    mesh = Mesh(np.asarray(devices), ("core",))
    spec = NamedSharding(mesh, PartitionSpec("core"))
    n_args = n_params + len(zero_outs)
    sharded = jax.jit(
        shard_map(_body, mesh=mesh,
                  in_specs=(PartitionSpec("core"),) * n_args,
                  out_specs=(PartitionSpec("core"),) * len(out_names),
                  check_rep=False),
        donate_argnums=donate, keep_unused=True)

    def stage(shards):
        """shards: list of NC_ per-core numpy arrays -> global device array."""
        arrs = [jax.device_put(shards[c], devices[c]) for c in range(NC_)]
        gshape = (NC_ * shards[0].shape[0],) + tuple(shards[0].shape[1:])
        return jax.make_array_from_single_device_arrays(gshape, spec, arrs)

    def run(staged):
        """staged: dict name -> global device array (one shard per core)."""
        args = [staged[n] for n in in_names]
        args += [stage([z.copy() for _ in range(NC_)]) for z in zero_outs]
        out_arrs = sharded(*args)
        outs = []
        for i, name in enumerate(out_names):
            g = np.asarray(out_arrs[i]).reshape(NC_, *out_avals[i].shape)
            outs.append((name, g))
        return dict(outs)

    _PROG_CACHE["runner"] = (stage, run)
    return _PROG_CACHE["runner"]


# ---------------------------------------------------------------------------
# Host-side input prep: everything computed once, cast to scaled fp8 early,
# per-core slices are cheap fp8 copies.  Each finished input is handed to a
# background staging thread so axon transfer overlaps the remaining prep.
# ---------------------------------------------------------------------------

def _pow2(a, bits):
    """a * 2**bits for positive-exponent-safe f32 arrays, via exponent add."""
    a = np.ascontiguousarray(a, np.float32)
    return (a.view(np.uint32) + np.uint32(bits << 23)).view(np.float32)


def _q8(a, bits=5):
    return _pow2(a, bits).astype(E4NP)


def _grp3(wT, width):   # [L,1152,width] -> [L,3,128,3*width], any dtype
    return np.ascontiguousarray(
        wT.reshape(L, 3, 3, 128, width).transpose(0, 1, 3, 2, 4)
    ).reshape(L, 3, 128, 3 * width)


def kernel(**inputs):
    nc = _get_prog()
    stage, run = _get_runner(nc)

    inp = {k: np.asarray(v) for k, v in inputs.items()}
    p = int(inp['position_ids'][0])
    tok = int(inp['input_ids'][0])
    assert p + 1 <= SEFF, f"position {p} exceeds compiled kv window {SEFF}"
    f32 = np.float32

    staged = {}
    errs = []
    q = _queue.Queue()

    def _stager():
        while True:
            item = q.get()
            if item is None:
                return
            name, shards = item
            try:
                staged[name] = stage(shards)
            except Exception as e:  # surfaced after join
                errs.append(e)

    th = threading.Thread(target=_stager, daemon=True)
    th.start()

    def put(name, shards):
        q.put((name, shards))

    def put_rep(name, arr):
        put(name, [arr] * NC_)

    # --- small tensors ---
    h0 = (inp['embed'][tok].astype(f32) * f32(HID ** 0.5)).reshape(1, HID)
    put_rep("h0row", h0)

    def sinsig(s):
        return np.concatenate([-s[0:128], s[128:256]])

    cs = np.concatenate([
        inp['cos_sliding'][p], sinsig(inp['sin_sliding'][p]),
        inp['cos_full'][p], sinsig(inp['sin_full'][p])]).astype(f32).reshape(1, 1024)
    put_rep("cs", cs)

    cm = inp['causal_mask'][:SEFF].astype(f32)
    um = inp['update_mask'][:SEFF, 0].astype(f32)
    col = lambda a: np.ascontiguousarray(a.reshape(T, 128).T)
    addm, umc = col(cm), col(um)
    vm = (addm > -1.0).astype(f32)
    mcol = np.concatenate([addm, vm, vm * (1 - umc), 1 - umc, umc],
                          axis=1).astype(f32)
    put_rep("mcol", mcol)

    # --- kv cache (replicated, x32 fp8, DMA-contiguous layouts) ---
    K8 = _q8(inp['kv_cache'][0:L, 0, 0:SEFF, :])            # [L,S,D]
    kt = np.ascontiguousarray(
        K8.transpose(0, 2, 1).reshape(L, 2, 128, SEFF).transpose(0, 2, 1, 3))
    put_rep("kt", kt)                                        # [L,128,2,S]
    V8 = _q8(inp['kv_cache'][L:2 * L, 0, 0:SEFF, :])
    vc = np.ascontiguousarray(
        V8.reshape(L, T, 128, D).transpose(0, 2, 1, 3))      # [L,128,T,D]
    put_rep("vc", vc)

    # --- attention weights (4 distinct heads, pairs replicate) ---
    Wq8 = _q8(inp['Wq'])                                     # [L,HD,HID]
    Wk8 = _q8(inp['Wk'])
    Wv8 = _q8(inp['Wv'])
    WkT = Wk8.transpose(0, 2, 1)
    WvT = Wv8.transpose(0, 2, 1)
    wqkv_h = []
    for hd in range(4):
        wcat = np.concatenate(
            [Wq8[:, hd * D:(hd + 1) * D, :].transpose(0, 2, 1), WkT, WvT],
            axis=2)                                          # [L,1152,768]
        wqkv_h.append(_grp3(wcat, 768))
    put("wqkv", [wqkv_h[c % 4] for c in range(NC_)])

    Wo8 = _q8(inp['Wo'], 4)                                  # x16: fp8 x32 * 0.5
    wo_h = []
    for hd in range(4):
        woT = np.ascontiguousarray(
            Wo8[:, :, hd * D:(hd + 1) * D].transpose(0, 2, 1)
        ).reshape(L, 2, 128, HID).transpose(0, 2, 1, 3)      # [L,128,2,HID]
        wo_h.append(np.ascontiguousarray(woT))
    put("wo", [wo_h[c % 4] for c in range(NC_)])

    # --- ffn weights (8-way over FF) ---
    WgT = _q8(inp['Wg']).transpose(0, 2, 1)                  # [L,HID,FF] fp8
    put("wg", [_grp3(np.ascontiguousarray(WgT[:, :, c * FSH:(c + 1) * FSH]),
                     FSH) for c in range(NC_)])
    WuT = _q8(inp['Wu']).transpose(0, 2, 1)
    put("wu", [_grp3(np.ascontiguousarray(WuT[:, :, c * FSH:(c + 1) * FSH]),
                     FSH) for c in range(NC_)])
    WdT = np.ascontiguousarray(_q8(inp['Wd']).transpose(0, 2, 1))  # [L,FF,HID]
    wd_shards = []
    for c in range(NC_):
        wdT = np.zeros((L, 896, HID), E4NP)
        wdT[:, :FSH, :] = WdT[:, c * FSH:(c + 1) * FSH, :]
        wd_shards.append(np.ascontiguousarray(
            wdT.reshape(L, 7, 128, HID).transpose(0, 2, 1, 3)))  # [L,128,7,HID]
    put("wd", wd_shards)

    # --- lm head (8-way over vocab) ---
    lm8 = _q8(inp['lm_head'])                                # [V,HID] fp8
    put("lm", [np.ascontiguousarray(
        lm8[c * VS:(c + 1) * VS, :].T).reshape(NCH, 128, VS)
        for c in range(NC_)])

    q.put(None)
    th.join()
    if errs:
        raise errs[0]

    res = run(staged)
    allg = np.concatenate([res["logits"][i][0] for i in range(NC_)])
    idx = int(np.argmax(allg))
    return np.int32(idx), np.float32(allg[idx])
